# revision 1
# baseline (speedup 1.0000x reference)
"""Trainium2 Bass kernel for nn_LogReg_455266533602 (histogram_binning).

Math: out[b] = sum_t W[0, text[t, b]] + bias -- the [B,V] histogram times W
collapses to a gather-and-reduce; the histogram is never materialized.

Hybrid kernel: gpsimd ap_gather path (A) + PE/DVE radix-256 one-hot path (B),
data-parallel over the batch across 8 NeuronCores (1024 phrases each).

Per core (1024 phrases):
  A-path (nA phrases/group x 8 groups): segmented-table ap_gather as in v1.
  B-path (remaining 8*(128-nA) phrases): for each token v = c*256 + a,
    G[c, n] = sum_k W2dT[k, c] * onehot_a[k, n]  (2 fp32 PE matmuls, k over 256)
    value = G[c_n, n] -> host-sent c-one-hot mask multiply + segmented reduce.
"""
import numpy as np

import concourse.bacc as bacc
import concourse.mybir as mybir
import concourse.tile as tile
from concourse.bass_utils import run_bass_kernel_spmd

P = 128
NCORES = 8
SEQ = 100
BPC = 1024             # phrases per core
NGRP = 8
SEG = 2048             # A-path table entries per partition
NSEG = 16
VPAD = SEG * NSEG      # 32768
V = 32000

# ---- split ----
NA = 72                # A-phrases per group (must be %4)
NB_PHR = (P - NA) * NGRP          # B phrases per core = 416
NI_A = NA * SEQ                   # idxs per group (A) = 7600
NB = NB_PHR * SEQ                 # B tokens per core = 41600

# A chunking: pieces of idxs per group, each %16 (wrap), %100 (phrases), and
# with 32-idx-aligned offsets (the Q7 ucode loads idxs as 32-bit words, so the
# int16 idx slice offset must be 4-byte aligned).
A_CHUNKS = [800] * (NI_A // 800) + ([NI_A % 800] if NI_A % 800 else [])
assert sum(A_CHUNKS) == NI_A and all(c % 400 == 0 for c in A_CHUNKS)
_off = 0
for _c in A_CHUNKS:
    assert (_off // 16) % 2 == 0
    _off += _c

# B chunking: token columns per chunk; %400 (PSUM matmul pieces of 400 = 4 phrases)
B_NT = 1200
B_CHUNKS = [B_NT] * (NB // B_NT) + ([NB % B_NT] if NB % B_NT else [])
assert all(c % 400 == 0 for c in B_CHUNKS)

RAD = 256              # radix for a = v & 255
NC_HI = 125            # c = v >> 8 in [0, 125)

F32 = mybir.dt.float32
I16 = mybir.dt.int16
I8 = mybir.dt.int8
U8 = mybir.dt.uint8

_cached = None


def _build():
    nc = bacc.Bacc("TRN2", debug=False)
    d_table = nc.declare_dram_parameter("table", [P, SEG], F32, isOutput=False)
    d_idx = nc.declare_dram_parameter("idx", [P, NI_A // 16], I16, isOutput=False)
    d_maska = nc.declare_dram_parameter("maska", [P, NI_A], I8, isOutput=False)
    d_ones8 = nc.declare_dram_parameter("ones8", [P, NGRP], F32, isOutput=False)
    d_brep = nc.declare_dram_parameter("brep", [NGRP, 1], F32, isOutput=False)
    d_w2dt = nc.declare_dram_parameter("w2dt", [P, 2 * NC_HI], F32, isOutput=False)
    d_abc = nc.declare_dram_parameter("abc", [P, NB], U8, isOutput=False)
    d_maskc = nc.declare_dram_parameter("maskc", [NC_HI, NB], I8, isOutput=False)
    d_iotal = nc.declare_dram_parameter("iotal", [P, 1], F32, isOutput=False)
    d_iotah = nc.declare_dram_parameter("iotah", [P, 1], F32, isOutput=False)
    d_ones125 = nc.declare_dram_parameter("ones125", [NC_HI, 1], F32, isOutput=False)
    d_brep1 = nc.declare_dram_parameter("brep1", [1, 1], F32, isOutput=False)
    d_outa = nc.declare_dram_parameter("outa", [NGRP, NA], F32, isOutput=True)
    d_outb = nc.declare_dram_parameter("outb", [1, NB_PHR], F32, isOutput=True)

    with tile.TileContext(nc) as tc:
        with (
            tc.tile_pool(name="const", bufs=1) as cpool,
            tc.tile_pool(name="g", bufs=len(A_CHUNKS)) as gpool,
            tc.tile_pool(name="p", bufs=2) as ppool,
            tc.tile_pool(name="oh", bufs=2) as ohpool,
            tc.tile_pool(name="pb", bufs=2) as pbpool,
            tc.tile_pool(name="psg", bufs=2, space="PSUM") as psg,
            tc.tile_pool(name="psf", bufs=1, space="PSUM") as psf,
        ):
            t_table = cpool.tile([P, SEG], F32)
            t_idx = cpool.tile([P, NI_A // 16], I16)
            t_maska = cpool.tile([P, NI_A], I8)
            t_ones8 = cpool.tile([P, NGRP], F32)
            t_brep = cpool.tile([NGRP, 1], F32)
            t_w2dt = cpool.tile([P, 2 * NC_HI], F32)
            t_abc = cpool.tile([P, NB], U8)
            t_maskc = cpool.tile([NC_HI, NB], I8)
            t_iotal = cpool.tile([P, 1], F32)
            t_iotah = cpool.tile([P, 1], F32)
            t_ones125 = cpool.tile([NC_HI, 1], F32)
            t_brep1 = cpool.tile([1, 1], F32)
            t_sa = cpool.tile([P, NA], F32)
            t_sb = cpool.tile([NC_HI, NB_PHR], F32)
            # DMA order matters for the Tile scheduler's simulated readiness:
            # B inputs (abc/maskc) early so B work heads the DVE/PE queues;
            # maska last so the A-path selects sort to the tail.
            for dst, src in ((t_table, d_table), (t_idx, d_idx),
                             (t_abc, d_abc), (t_maskc, d_maskc),
                             (t_w2dt, d_w2dt), (t_iotal, d_iotal),
                             (t_iotah, d_iotah), (t_ones125, d_ones125),
                             (t_ones8, d_ones8), (t_brep, d_brep),
                             (t_brep1, d_brep1), (t_maska, d_maska)):
                nc.sync.dma_start(out=dst[:], in_=src[:])

            # ---------------- A + B interleaved ----------------
            def a_chunk(off, ch):
                t_g = gpool.tile([P, max(A_CHUNKS)], F32, tag="ga")
                nc.gpsimd.ap_gather(
                    out_ap=t_g[:, :ch],
                    in_ap=t_table[:],
                    idxs_ap=t_idx[:, off // 16:(off + ch) // 16],
                    channels=P, num_elems=SEG, d=1, num_idxs=ch)
                t_p = ppool.tile([P, max(A_CHUNKS)], F32, tag="pa")
                nc.vector.tensor_tensor(
                    out=t_p[:, :ch], in0=t_g[:, :ch],
                    in1=t_maska[:, off:off + ch],
                    op=mybir.AluOpType.mult)
                nc.vector.tensor_reduce(
                    out=t_sa[:, off // SEQ:(off + ch) // SEQ],
                    in_=t_p[:, :ch].rearrange("p (b t) -> p b t", t=SEQ),
                    axis=mybir.AxisListType.X,
                    op=mybir.AluOpType.add)

            def b_chunk(off, ch):
                t_ohl = ohpool.tile([P, B_NT], F32, tag="ohl")
                t_ohh = ohpool.tile([P, B_NT], F32, tag="ohh")
                nc.vector.tensor_scalar(
                    out=t_ohl[:, :ch], in0=t_abc[:, off:off + ch],
                    scalar1=t_iotal[:], scalar2=None,
                    op0=mybir.AluOpType.is_equal)
                nc.vector.tensor_scalar(
                    out=t_ohh[:, :ch], in0=t_abc[:, off:off + ch],
                    scalar1=t_iotah[:], scalar2=None,
                    op0=mybir.AluOpType.is_equal)
                t_gb = psg.tile([NC_HI, B_NT], F32, tag="gb")
                pieces = [(q, min(512, ch - q)) for q in range(0, ch, 512)]
                for q, w in pieces:
                    nc.tensor.matmul(out=t_gb[:, q:q + w],
                                     lhsT=t_w2dt[:, :NC_HI],
                                     rhs=t_ohl[:, q:q + w],
                                     start=True, stop=False)
                for q, w in pieces:
                    nc.tensor.matmul(out=t_gb[:, q:q + w],
                                     lhsT=t_w2dt[:, NC_HI:],
                                     rhs=t_ohh[:, q:q + w],
                                     start=False, stop=True)
                t_pb = pbpool.tile([NC_HI, B_NT], F32, tag="pb")
                nc.vector.tensor_tensor(
                    out=t_pb[:, :ch], in0=t_gb[:, :ch],
                    in1=t_maskc[:, off:off + ch],
                    op=mybir.AluOpType.mult)
                nc.vector.tensor_reduce(
                    out=t_sb[:, off // SEQ:(off + ch) // SEQ],
                    in_=t_pb[:, :ch].rearrange("p (b t) -> p b t", t=SEQ),
                    axis=mybir.AxisListType.X,
                    op=mybir.AluOpType.add)

            # B first in program order: the per-engine static schedules then
            # let PE/DVE chew through all B work while gpsimd gathers run;
            # the A-path selects land at the tail. gpool has one buf per
            # gather so the gathers never wait on the (late) A selects.
            off = 0
            for ch in B_CHUNKS:
                b_chunk(off, ch); off += ch
            off = 0
            for ch in A_CHUNKS:
                a_chunk(off, ch); off += ch

            # ---------------- finals ----------------
            t_psa = psf.tile([NGRP, NA], F32)
            nc.tensor.matmul(out=t_psa[:], lhsT=t_ones8[:], rhs=t_sa[:],
                             start=True, stop=True)
            t_oa = cpool.tile([NGRP, NA], F32)
            nc.vector.tensor_scalar(
                out=t_oa[:], in0=t_psa[:], scalar1=t_brep[:], scalar2=None,
                op0=mybir.AluOpType.add)
            nc.sync.dma_start(out=d_outa[:], in_=t_oa[:])

            t_psb = psf.tile([1, NB_PHR], F32)
            nc.tensor.matmul(out=t_psb[:], lhsT=t_ones125[:], rhs=t_sb[:],
                             start=True, stop=True)
            t_ob = cpool.tile([1, NB_PHR], F32)
            nc.vector.tensor_scalar(
                out=t_ob[:], in0=t_psb[:], scalar1=t_brep1[:], scalar2=None,
                op0=mybir.AluOpType.add)
            nc.sync.dma_start(out=d_outb[:], in_=t_ob[:])
    nc.compile()
    return nc


def _prep_inputs(text: np.ndarray, W: np.ndarray, b: np.ndarray):
    wpad = np.zeros(VPAD, np.float32)
    wpad[:V] = W[0].astype(np.float32)
    table = np.tile(wpad.reshape(NSEG, SEG), (NGRP, 1))
    ones8 = np.repeat(np.eye(NGRP, dtype=np.float32), 16, axis=0)
    brep = np.full((NGRP, 1), np.float32(b[0]), np.float32)
    # W2dT[k, c] = W[c*256 + k]; two halves side by side [128, 250]
    w2d = wpad[:V].reshape(NC_HI, RAD)      # [125, 256] row c holds W[c*256 : ...]
    w2dt = np.concatenate([w2d[:, :P].T, w2d[:, P:].T], axis=1)  # [128, 250]
    w2dt = np.ascontiguousarray(w2dt)
    iotal = np.arange(P, dtype=np.float32).reshape(P, 1)
    iotah = (np.arange(P, dtype=np.float32) + P).reshape(P, 1)
    ones125 = np.ones((NC_HI, 1), np.float32)
    brep1 = np.full((1, 1), np.float32(b[0]), np.float32)

    in_maps = []
    for c in range(NCORES):
        v = np.ascontiguousarray(text[:, c * BPC:(c + 1) * BPC]).astype(np.int64)
        vp = v.T                                   # [1024, 100] phrase-major
        # ---- A: first NA phrases of each 128-block ----
        vA = vp.reshape(NGRP, P, SEQ)[:, :NA, :].reshape(NGRP, NI_A)
        oA = (vA & (SEG - 1)).astype(np.int16)
        sA = (vA >> 11).astype(np.int64)
        idx = oA.reshape(NGRP, NI_A // 16, 16).transpose(0, 2, 1).reshape(P, NI_A // 16)
        maska = np.zeros((NGRP, NSEG, NI_A), np.int8)
        maska[np.arange(NGRP)[:, None], sA, np.arange(NI_A)[None, :]] = 1
        # ---- B: remaining phrases ----
        vB = vp.reshape(NGRP, P, SEQ)[:, NA:, :].reshape(NB)
        aB = (vB & (RAD - 1)).astype(np.uint8)
        cB = (vB >> 8).astype(np.int64)
        abc = np.broadcast_to(aB[None, :], (P, NB))
        maskc = np.zeros((NC_HI, NB), np.int8)
        maskc[cB, np.arange(NB)] = 1
        in_maps.append({
            "table": table, "idx": np.ascontiguousarray(idx),
            "maska": maska.reshape(P, NI_A), "ones8": ones8, "brep": brep,
            "w2dt": w2dt, "abc": np.ascontiguousarray(abc), "maskc": maskc,
            "iotal": iotal, "iotah": iotah, "ones125": ones125, "brep1": brep1,
        })
    return in_maps


def kernel(text: np.ndarray, W: np.ndarray, b: np.ndarray) -> np.ndarray:
    global _cached
    if _cached is None:
        _cached = _build()
    nc = _cached
    in_maps = _prep_inputs(np.asarray(text), np.asarray(W), np.asarray(b))
    res = run_bass_kernel_spmd(nc, in_maps, list(range(NCORES)))
    full = np.empty((NCORES, BPC), np.float32)
    for c in range(NCORES):
        oa = res.results[c]["outa"].reshape(NGRP, NA)
        ob = res.results[c]["outb"].reshape(NGRP, P - NA)
        blk = np.concatenate([oa, ob], axis=1)    # [8, 128] in phrase order
        full[c] = blk.reshape(BPC)
    return full.reshape(NCORES * BPC, 1).astype(np.float32)


if __name__ == "__main__":
    rng = np.random.default_rng(0)
    text = rng.integers(0, V, size=(SEQ, BPC * NCORES)).astype(np.int64)
    W = rng.standard_normal((1, V)).astype(np.float32)
    b = np.zeros(1, np.float32)
    got = kernel(text, W, b)
    exp = (W[0][text].sum(axis=0) + b[0]).reshape(-1, 1).astype(np.float32)
    err = np.abs(got - exp).max() / np.abs(exp).max()
    print("max abs rel err:", err)
    print("OK" if err < 1e-5 else "FAIL")

